# revision 31
# baseline (speedup 1.0000x reference)
"""Trainium2 Bass kernel v6 for nn_Attention_22050362097758 (edge-biased causal
attention; b=2, n=512, dim=256, heads=8, dim_head=64).

Sharding: core c -> batch c//4, lane l=c%4, query rows i = 4r+l (r=0..127).
Row-interleaving makes causal extents core-uniform (SPMD: one program).

Only the d-on-partitions fp8 causal-prefix edge pack is loaded (10.5 MB/core).
The sum-of-squares rides the PE: ACT/GpSimd/DVE square the d-layout slabs in
8-tile sub-batches (per SS_MODE, split by measured engine rates), then
per-slab ones-matmuls reduce over d with the squared slab as FWL weights,
depositing ss next to the bias columns (bias_chunk[:, q, 8]). Per chunk all
64 bias matmuls are emitted before the ss matmuls so the PE starts on DMA
arrival and overlaps the squares. Tiles are t-major (chunk == one j-block
slice), chunks ascend, and each sim block is split in two phases pipelined
across chunk iterations: A (rinv + staging, ACT/DVE) right at the trigger, B
(sim matmuls + scr + exp + attn@[v|1]) one chunk later. Staging is h-major
so the stgtmp/stgfin ops hit the DVE 2x_1P packed mode (the stride-0
broadcast against t-major staging ran at ~1/6 speed). Sim matmuls use
zero-padded K=128 operands (kTz/qTz) -- K=64 matmuls at base partitions 0/64
run concurrently on different PE row-groups and collide draining into one
PSUM bank (hangs HW). den rides the attn@v matmul as a ones-column on V; av
accumulates across blocks in SBUF (PSUM allows one pending accumulation
group per bank).
"""
import sys
sys.path.insert(0, "/opt/trn_rl_repo")
import numpy as np
import ml_dtypes

import concourse.bass as bass
import concourse.mybir as mybir
import concourse.tile as tile
from concourse.bass_utils import run_bass_kernel_spmd

B, N, DIM = 2, 512, 256
H, DH = 8, 64
EPS = 1e-5
NEG = -1e30
F32 = mybir.dt.float32
F16 = mybir.dt.float16
BF16 = mybir.dt.bfloat16
FP8 = mybir.dt.float8e4
BF = ml_dtypes.bfloat16
F8 = ml_dtypes.float8_e4m3fn

# t-major tile order: global tile index k = POS(r, t) = TOFF[t] + (r - 32*t).
TOFF = [0, 128, 224, 288, 320]
NR = [128, 96, 64, 32]
NT = 320
CH = 32
NCH = NT // CH   # 10
CHUNK_T = [0, 0, 0, 0, 1, 1, 1, 2, 2, 3]
SUB = 8          # square sub-batch (tiles per square op)


def POS(r, t):
    return TOFF[t] + (r - 32 * t)


# per-chunk square engine: 'a'=ACT Square (sub-batched), 'h'=hybrid
# GpSimd(16 tiles) + DVE(16 tiles) -- whole-chunk GpSimd squares (~14 us)
# stall the PE pipeline; halves fit the ~6 us chunk cadence.
SS_MODE = "ahhahhahha"
assert len(SS_MODE) == NCH

_ctr = [0]


def _nop_with_wait(engine, wait):
    _ctr[0] += 1
    n = mybir.InstNoOp.__new__(mybir.InstNoOp, name=f"waitnop-{_ctr[0]}")
    n.engine = engine
    n.sync_info = mybir.SyncInfo.__new__(mybir.SyncInfo, on_wait=[wait], on_update=[])
    return n


def split_waits(nc):
    """Walrus encodes at most ONE sem-wait per instruction; Tile attaches
    many. Move extras onto NOPs inserted just before, same engine."""
    for f in nc.m.functions:
        for b in f.blocks:
            out, changed = [], False
            for inst in b.instructions:
                si = inst.sync_info
                waits = list(si.on_wait) if (si and si.on_wait) else []
                keep = 0 if inst.opcode == "Drain" else 1
                if len(waits) > keep:
                    changed = True
                    moved = waits[:-keep] if keep else waits
                    kept = waits[-keep:] if keep else []
                    for w in moved:
                        out.append(_nop_with_wait(inst.engine, w))
                    inst.sync_info = mybir.SyncInfo.__new__(
                        mybir.SyncInfo, on_wait=kept,
                        on_update=list(si.on_update) if si.on_update else [])
                out.append(inst)
            if changed:
                b.instructions = out


def build(debug=False):
    nc = bass.Bass()
    ef8_ext = nc.declare_dram_parameter("ef8", [128, NCH, CH, DIM], FP8,
                                        isOutput=False)
    x_ext = nc.declare_dram_parameter("xb", [N, DIM], F32, isOutput=False)
    xq_ext = nc.declare_dram_parameter("xq", [128, DIM], F32, isOutput=False)
    wq32_ext = nc.declare_dram_parameter("wq32", [128, 2, 8, 128], F32, isOutput=False)
    wv16_ext = nc.declare_dram_parameter("wv16", [128, 2, 4, 128], BF16, isOutput=False)
    we16_ext = nc.declare_dram_parameter("we16", [128, 2, H], BF16, isOutput=False)
    wo16_ext = nc.declare_dram_parameter("wo16", [128, 4, DIM], BF16, isOutput=False)
    mcolh_ext = nc.declare_dram_parameter("mcolh", [128, H, NT], BF16, isOutput=False)
    id_ext = nc.declare_dram_parameter("ident", [128, 128], BF16, isOutput=False)
    out_ext = nc.declare_dram_parameter("out", [128, DIM], F32, isOutput=True)
    if debug:
        dbg_ss = nc.declare_dram_parameter("dbg_ss", [128, NT], F32, isOutput=True)
        dbg_av = nc.declare_dram_parameter("dbg_av", [128, H * DH], F32, isOutput=True)
        dbg_attn = nc.declare_dram_parameter("dbg_attn", [128, 4, H, 128], F32,
                                             isOutput=True)

    AF = mybir.ActivationFunctionType
    MUL, ADD = mybir.AluOpType.mult, mybir.AluOpType.add

    with tile.TileContext(nc) as tc:
        with tc.tile_pool(name="cst", bufs=1) as cst, \
             tc.tile_pool(name="ep", bufs=3) as ep, \
             tc.tile_pool(name="sqp", bufs=3) as sqp, \
             tc.tile_pool(name="wk", bufs=2) as wk, \
             tc.tile_pool(name="bps", bufs=2, space="PSUM") as bps, \
             tc.tile_pool(name="sps", bufs=1, space="PSUM") as sps, \
             tc.tile_pool(name="mps", bufs=2, space="PSUM") as mps, \
             tc.tile_pool(name="avps", bufs=1, space="PSUM") as avps:

            # ---------------- constants ----------------
            ident = cst.tile([128, 128], BF16)
            nc.sync.dma_start(out=ident, in_=id_ext[:, :])
            we16 = cst.tile([128, 2, H], BF16)
            nc.sync.dma_start(out=we16, in_=we16_ext[:, :, :])
            wo16 = cst.tile([128, 4, DIM], BF16)
            nc.sync.dma_start(out=wo16, in_=wo16_ext[:, :, :])
            wq32 = cst.tile([128, 2, 8, 128], F32)
            nc.sync.dma_start(out=wq32, in_=wq32_ext[:, :, :, :])
            wv16 = cst.tile([128, 2, 4, 128], BF16)
            nc.sync.dma_start(out=wv16, in_=wv16_ext[:, :, :, :])
            mcolh = cst.tile([128, H, NT], BF16)
            nc.sync.dma_start(out=mcolh, in_=mcolh_ext[:, :, :])
            x32 = cst.tile([128, 5, DIM], F32)
            nc.sync.dma_start(out=x32[:, 0:4, :],
                              in_=x_ext.rearrange("(t p) d -> p t d", p=128))
            nc.sync.dma_start(out=x32[:, 4, :], in_=xq_ext[:, :])

            epsc = cst.tile([128, 1], F32)
            nc.vector.memset(epsc, EPS)
            ones16 = cst.tile([128, 1], BF16)
            nc.vector.memset(ones16, 1.0)

            ss_all = cst.tile([128, NT], F32)
            # stgraw: h-major staging [j, h, rows]
            stgraw = []
            for t in range(4):
                sr = cst.tile([128, H, NR[t]], BF16, tag=f"sraw{t}", name=f"sraw{t}")
                stgraw.append(sr)

            # ---------------- x path: rmsnorm, kTz, qTz, v ----------------
            ssx = wk.tile([128, 5], F32, tag="ssx")
            for t in range(5):
                dump = wk.tile([128, DIM], BF16, tag="sqdump")
                nc.scalar.activation(out=dump, in_=x32[:, t, :],
                                     func=AF.Square, accum_out=ssx[:, t:t + 1])
            sqm = wk.tile([128, 5], F32, tag="sqm")
            nc.scalar.activation(out=sqm, in_=ssx, func=AF.Ln,
                                 bias=epsc, scale=1.0 / DIM)
            rx = wk.tile([128, 5], F32, tag="rx")
            nc.scalar.activation(out=rx, in_=sqm, func=AF.Exp, scale=-0.5)
            xn32 = cst.tile([128, 5, DIM], F32)
            for t in range(5):
                nc.scalar.activation(out=xn32[:, t, :], in_=x32[:, t, :],
                                     func=AF.Copy, scale=rx[:, t:t + 1])
            ident32 = cst.tile([128, 128], F32)
            nc.vector.tensor_copy(ident32, ident)
            xnT32 = cst.tile([128, 2, 5, 128], F32)
            for t in range(5):
                ps32 = mps.tile([128, 2, 128], F32, tag="setup_ps")
                for kh in range(2):
                    nc.tensor.transpose(ps32[:, kh, :],
                                        xn32[:, t, kh * 128:(kh + 1) * 128], ident32)
                nc.scalar.copy(xnT32[:, :, t, :], ps32)
            xnT = cst.tile([128, 2, 5, 128], BF16)
            nc.vector.tensor_copy(xnT, xnT32)

            # kTz/qTz: per-head K=128 operands, zero-padded on the other
            # head's 64 partitions. K=64 matmuls at base partitions 0/64 run
            # CONCURRENTLY on different PE row-groups and collide draining
            # into one PSUM bank (hangs HW); full-K serializes the array.
            kTz = cst.tile([128, H, N], BF16)
            nc.gpsimd.memset(kTz, 0.0)
            qTz = cst.tile([128, H, 128], BF16)
            nc.gpsimd.memset(qTz, 0.0)
            for ft in range(4):
                k_ps = mps.tile([128, N], F32, tag="setup_ps")
                for kh in range(2):
                    nc.tensor.matmul(k_ps,
                                     lhsT=wq32[:, kh, 4 + ft, :],
                                     rhs=xnT32[:, kh, 0:4, :].rearrange("p a b -> p (a b)"),
                                     start=(kh == 0), stop=(kh == 1))
                for s in range(2):
                    nc.scalar.copy(kTz[64 * s:64 * (s + 1), 2 * ft + s, :],
                                   k_ps[64 * s:64 * (s + 1), :])
            for ft in range(4):
                q_ps = mps.tile([128, 128], F32, tag="setup_ps")
                for kh in range(2):
                    nc.tensor.matmul(q_ps, lhsT=wq32[:, kh, ft, :],
                                     rhs=xnT32[:, kh, 4, :],
                                     start=(kh == 0), stop=(kh == 1))
                for s in range(2):
                    nc.scalar.copy(qTz[64 * s:64 * (s + 1), 2 * ft + s, :],
                                   q_ps[64 * s:64 * (s + 1), :])
            # V with a ones-column appended per head: attn @ [v | 1] gives
            # av in cols 0:64 and the softmax denominator in col 64.
            v16e = cst.tile([128, 4, H, DH + 1], BF16)
            nc.gpsimd.memset(v16e, 1.0)
            for st in range(4):
                v_ps = mps.tile([128, H * DH], F32, tag="setup_ps")
                for kh in range(2):
                    nc.tensor.matmul(v_ps,
                                     lhsT=xnT[:, kh, st, :],
                                     rhs=wv16[:, kh, :, :].rearrange("p a b -> p (a b)"),
                                     start=(kh == 0), stop=(kh == 1))
                nc.scalar.copy(v16e[:, st, :, 0:DH],
                               v_ps.rearrange("p (h d) -> p h d", h=H))

            # av accumulates across blocks in SBUF (PSUM allows only one
            # pending accumulation group per bank).
            avacc = cst.tile([128, H, DH + 2], F32)
            nc.gpsimd.memset(avacc, 0.0)
            # attn tiles: [j, t, h, i]; masked region stays 0 from this memset
            attn_all = cst.tile([128, 4, H, 128], BF16)
            nc.gpsimd.memset(attn_all, 0.0)

            # ------------- edges + attention, chunks ascending -------------
            stg_of = {}

            def block_phase_a(t):
                """rinv + h-major staging for block t (ACT + DVE)."""
                n = NR[t]
                sl = slice(TOFF[t], TOFF[t + 1])
                srt = wk.tile([128, 128], F32, tag="srt")
                nc.scalar.activation(out=srt[:, 0:n], in_=ss_all[:, sl],
                                     func=AF.Ln, bias=epsc, scale=1.0 / DIM)
                rinv = wk.tile([128, 128], BF16, tag="rinv")
                nc.scalar.activation(out=rinv[:, 0:n], in_=srt[:, 0:n],
                                     func=AF.Exp, scale=-0.5)
                stgtmp = wk.tile([128, H, 128], BF16, tag="stgtmp")
                nc.vector.tensor_mul(
                    stgtmp[:, :, 0:n], stgraw[t],
                    rinv[:, 0:n].rearrange("p (o n) -> p o n", o=1)
                    .broadcast_to([128, H, n]))
                stgfin = wk.tile([128, H, 128], BF16, tag="stgfin")
                nc.vector.tensor_add(stgfin[:, :, 0:n], stgtmp[:, :, 0:n],
                                     mcolh[:, :, sl])
                stg_of[t] = stgfin

            def block_phase_b(t):
                """sim matmuls + scr + exp + attn@[v|1] for block t."""
                n = NR[t]
                stgfin = stg_of.pop(t)
                sim_ps = sps.tile([128, H, 128], F32, tag="sim")
                for h in range(H):
                    nc.tensor.matmul(sim_ps[:, h, 0:n],
                                     lhsT=kTz[:, h, t * 128:(t + 1) * 128],
                                     rhs=qTz[:, h, 32 * t:128],
                                     start=True, stop=True)
                scr = wk.tile([128, H, 128], F32, tag="scr")
                nc.vector.tensor_add(scr[:, :, 0:n], sim_ps[:, :, 0:n],
                                     stgfin[:, :, 0:n])
                nc.scalar.activation(out=attn_all[:, t, :, 32 * t:],
                                     in_=scr[:, :, 0:n], func=AF.Exp)
                avb = []
                for g in range(2):
                    avb.append(avps.tile([128, 4, DH + 2], F32, tag=f"av{g}",
                                         name=f"avb{g}_{t}"))
                for h in range(H):
                    nc.tensor.matmul(avb[h // 4][:, h % 4, 0:DH + 1],
                                     lhsT=attn_all[:, t, h, :],
                                     rhs=v16e[:, t, h, :],
                                     start=True, stop=True)
                for g in range(2):
                    nc.vector.tensor_add(avacc[:, 4 * g:4 * g + 4, 0:DH + 1],
                                         avacc[:, 4 * g:4 * g + 4, 0:DH + 1],
                                         avb[g][:, :, 0:DH + 1])

            sqs = {}
            bcs = {}
            done_ss = set()

            def emit_ss_mms(c):
                sq, bias_chunk = sqs[c], bcs[c]
                for q in range(CH):
                    for kh in range(2):
                        nc.tensor.matmul(
                            bias_chunk[:, q, 8:9],
                            lhsT=sq[:, q, kh * 128:(kh + 1) * 128],
                            rhs=ones16,
                            start=(kh == 0), stop=(kh == 1))

            def emit_copies(c):
                tb = CHUNK_T[c]
                lo = 32 * c - TOFF[tb]
                bias_chunk = bcs.pop(c)
                del sqs[c]
                nc.vector.tensor_copy(
                    stgraw[tb][:, :, lo:lo + CH].rearrange("p h n -> p n h"),
                    bias_chunk[:, :, 0:8])
                nc.vector.tensor_copy(ss_all[:, 32 * c:32 * (c + 1)],
                                      bias_chunk[:, :, 8])

            for c in range(NCH):
                ec = ep.tile([128, CH, DIM], FP8, tag="ec")
                nc.scalar.dma_start(out=ec, in_=ef8_ext[:, c, :, :])
                mode = SS_MODE[c]
                bias_chunk = bps.tile([128, CH, 10], F32, tag="bias")
                bcs[c] = bias_chunk
                # pending chunks' ss matmuls first: their squares had 1-2
                # full chunk slots to finish (GpSimd squares are ~14 us, so
                # 'g' chunks get a 2-slot lead), so the PE doesn't stall
                for p in sorted(list(sqs)):
                    if p + 1 <= c:
                        emit_ss_mms(p)
                        done_ss.add(p)
                for q in range(CH):
                    for kh in range(2):
                        nc.tensor.matmul(bias_chunk[:, q, 0:8],
                                         lhsT=ec[:, q, kh * 128:(kh + 1) * 128],
                                         rhs=we16[:, kh, :],
                                         start=(kh == 0), stop=(kh == 1))
                sq = sqp.tile([128, CH, DIM],
                              FP8 if mode == "a" else BF16,
                              tag="sqa" if mode == "a" else "sqg")
                sqs[c] = sq
                if mode == "h":
                    nc.gpsimd.tensor_tensor(sq[:, 0:16, :], ec[:, 0:16, :],
                                            ec[:, 0:16, :], op=MUL)
                    for s0 in (16, 24):
                        nc.vector.tensor_tensor(sq[:, s0:s0 + 8, :],
                                                ec[:, s0:s0 + 8, :],
                                                ec[:, s0:s0 + 8, :], op=MUL)
                else:
                    for s0 in range(0, CH, SUB):
                        nc.scalar.activation(out=sq[:, s0:s0 + SUB, :],
                                             in_=ec[:, s0:s0 + SUB, :],
                                             func=AF.Square)
                for p in sorted(list(done_ss)):
                    emit_copies(p)
                    done_ss.discard(p)
                if c == 4:
                    block_phase_a(0)
                elif c == 5:
                    block_phase_b(0)
                elif c == 7:
                    block_phase_a(1)
                elif c == 8:
                    block_phase_b(1)
                elif c == 9:
                    for p in sorted(list(sqs)):
                        if p < 9:
                            emit_ss_mms(p)
                            emit_copies(p)
                    block_phase_a(2)
            emit_ss_mms(9)
            emit_copies(9)
            block_phase_b(2)
            block_phase_a(3)
            block_phase_b(3)

            # ---------------- epilogue ----------------
            rv = cst.tile([128, H], F32)
            nc.vector.reciprocal(rv, avacc[:, :, DH])
            av_sb = cst.tile([128, H * DH], BF16)
            for h in range(H):
                nc.vector.tensor_scalar(out=av_sb[:, h * DH:(h + 1) * DH],
                                        in0=avacc[:, h, 0:DH],
                                        scalar1=rv[:, h:h + 1], scalar2=None,
                                        op0=MUL)
            avT = cst.tile([128, 4, 128], BF16)
            nc.sync.dma_start(out=avT, in_=av_sb, transpose=True)
            out_ps = mps.tile([128, DIM], F32, tag="setup_ps")
            for q4 in range(4):
                nc.tensor.matmul(out_ps, lhsT=avT[:, q4, :], rhs=wo16[:, q4, :],
                                 start=(q4 == 0), stop=(q4 == 3))
            out_sb = cst.tile([128, DIM], F32)
            nc.vector.tensor_copy(out_sb, out_ps)
            nc.sync.dma_start(out=out_ext[:, :], in_=out_sb)
            if debug:
                nc.sync.dma_start(out=dbg_ss[:, :], in_=ss_all)
                av32 = cst.tile([128, H * DH], F32)
                nc.vector.tensor_copy(av32, av_sb)
                nc.sync.dma_start(out=dbg_av[:, :], in_=av32)
                at32 = cst.tile([128, 4, H, 128], F32)
                nc.vector.tensor_copy(at32, attn_all)
                nc.sync.dma_start(out=dbg_attn[:, :, :, :], in_=at32)
    return nc


_NC_CACHE = [None]
LAST_RESULT = [None]


def _pack_core(edges_b8, x, b, l, mask, b_edge):
    """Per-core host packing: fp8 causal-prefix edges, d-on-partitions,
    t-major tile order; plus the mask/b_edge tensor mcolh (h-major)."""
    E8 = edges_b8[l::4]                       # [128, 512, 256] fp8
    eT_all = np.empty((128, NT, DIM), dtype=F8)
    mcolh = np.empty((128, H, NT), np.float32)
    jj = np.arange(128)
    for t in range(4):
        nr = NR[t]
        blk = E8[32 * t:, 128 * t:128 * (t + 1), :]        # [nr, 128, 256]
        eT_all[:, TOFF[t]:TOFF[t] + nr, :] = (
            blk.transpose(2, 0, 1).reshape(2, 128, nr, 128)
            .transpose(1, 2, 0, 3).reshape(128, nr, DIM))
        r = np.arange(32 * t, 128)
        valid = (128 * t + jj[:, None] <= 4 * r[None, :] + l) \
            & mask[b, 128 * t + jj][:, None]               # [128, nr]
        mcolh[:, :, TOFF[t]:TOFF[t] + nr] = np.where(
            valid[:, None, :], b_edge[None, :, None], NEG)
    ef8 = np.ascontiguousarray(eT_all.reshape(128, NCH, CH, DIM))
    xq = np.ascontiguousarray(x[b, l::4])
    return ef8, mcolh.astype(BF), xq


def kernel(x, mask, edges, gamma_x, W_qkv, gamma_e, W_edge, b_edge, W_out):
    x = np.asarray(x, np.float32)
    mask = np.asarray(mask)
    edges = np.asarray(edges, np.float32)
    gamma_x = np.asarray(gamma_x, np.float32)
    W_qkv = np.asarray(W_qkv, np.float32)
    gamma_e = np.asarray(gamma_e, np.float32)
    W_edge = np.asarray(W_edge, np.float32)
    b_edge = np.asarray(b_edge, np.float32)
    W_out = np.asarray(W_out, np.float32)

    wqkv_f = (gamma_x[:, None] * W_qkv).copy()
    wqkv_f[:, :H * DH] *= DH ** 0.5
    wq32 = np.ascontiguousarray(
        wqkv_f[:, :1024].reshape(2, 128, 8, 128).transpose(1, 0, 2, 3))
    wv16 = np.ascontiguousarray(
        wqkv_f[:, 1024:1536].reshape(2, 128, 4, 128).transpose(1, 0, 2, 3)
    ).astype(BF)
    wedge_f = gamma_e[:, None] * W_edge
    we16 = np.ascontiguousarray(
        wedge_f.reshape(2, 128, H).transpose(1, 0, 2)).astype(BF)
    wo16 = np.ascontiguousarray(
        W_out.reshape(4, 128, DIM).transpose(1, 0, 2)).astype(BF)
    ident = np.eye(128, dtype=BF)

    # |e| <= 15 so e^2 <= 225 stays under TRN fp8e4's 240 max-normal
    edges8 = np.clip(edges, -15.0, 15.0).astype(F8)

    in_maps = []
    for c in range(8):
        b, l = c // 4, c % 4
        ef8, mcolh, xq = _pack_core(edges8[b], x, b, l, mask, b_edge)
        in_maps.append({
            "ef8": ef8, "xb": x[b], "xq": xq,
            "wq32": wq32, "wv16": wv16, "we16": we16, "wo16": wo16,
            "mcolh": mcolh, "ident": ident,
        })

    if _NC_CACHE[0] is None:
        nc = build()
        split_waits(nc)
        _NC_CACHE[0] = nc
    res = run_bass_kernel_spmd(_NC_CACHE[0], in_maps, core_ids=list(range(8)))
    LAST_RESULT[0] = res

    out = np.zeros((B, N, DIM), np.float32)
    for c in range(8):
        b, l = c // 4, c % 4
        out[b, l::4] = res.results[c]["out"]
    return out
